# revision 3
# baseline (speedup 1.0000x reference)
"""Trainium2 (8 NeuronCores) kernel for coverage attention — v5.

feat.T layout, host-folded coverage, host-transposed x, bf16 matmuls.
All instruction types are hardware-proven (v1 vocabulary): matmul with
PSUM accumulate, activation with per-partition bias reading PSUM,
tensor_copy, tensor_reduce, DMA.

Per-core pipeline, for each (batch b, 512-seq group g):
  - DMA x~T slab [128h, 4k, 512s] bf16  (x~ = x + cov (x) u folded on
    host via u = Wc[0] @ Wh^{-1}, pre-transposed to [H, S] per batch)
  - for m in 4: PSUM[128 h_m, 512 s] = sum_k Wh[k, ms].T @ x~T_k
    (4 bf16 matmuls, 512 cyc each); tanh via ACT with bias
    A.T[ms, b] (per-partition — free); e chunk accumulated on PE:
    ps_e[1, 512] += vw[ms].T @ f_m  (bf16)
  - e row -> e_sb[b] via partition-0 scratch + SB->SB DMA (v1 pattern)
Epilogue: v1's row softmax on [bpc, S] (max-reduce, Exp w/ accum, recip,
scale).  sum_coverage = coverage + a_t on host.

Engine budget (cost model): PE 137us (20x512cyc/group), ACT 101us,
DVE ~30us, DMA ~55us.
"""

import os
import sys

for _p in ("/opt/trn_rl_repo", os.path.expanduser("~/.axon_site/_ro/trn_rl_repo")):
    if os.path.isdir(_p) and _p not in sys.path:
        sys.path.insert(0, _p)

import numpy as np

import concourse.bass as bass
from concourse import bacc
import concourse.tile as tile
from concourse import mybir

B, S, H = 64, 2048, 512
N_CORES = 8
BPC = B // N_CORES

FP = mybir.dt.float32
BF = mybir.dt.bfloat16

SLAB = 512
JT = SLAB // 128
NG = S // SLAB
HC = H // 128


def build_program(bpc=BPC, s=S):
    nc = bacc.Bacc(None)

    xt = nc.declare_dram_parameter("xt", [bpc * H, s], BF, isOutput=False)
    wh = nc.declare_dram_parameter("wh", [H, H], BF, isOutput=False)
    at = nc.declare_dram_parameter("at", [H, bpc], FP, isOutput=False)
    vwt = nc.declare_dram_parameter("vwt", [128, HC], BF, isOutput=False)
    out_a = nc.declare_dram_parameter("out_a", [bpc, s], FP, isOutput=True)

    from contextlib import ExitStack
    with tile.TileContext(nc) as tc, ExitStack() as ctx:
        const = ctx.enter_context(tc.tile_pool(name="const", bufs=1))
        xpool = ctx.enter_context(tc.tile_pool(name="xpool", bufs=3))
        fpool = ctx.enter_context(tc.tile_pool(name="fpool", bufs=9))
        egpool = ctx.enter_context(tc.tile_pool(name="egpool", bufs=3))
        psf_pool = ctx.enter_context(tc.tile_pool(name="ps_f", bufs=5, space="PSUM"))
        pse_pool = ctx.enter_context(tc.tile_pool(name="ps_e", bufs=3, space="PSUM"))

        # ---------------- preamble ----------------
        wh_sb = []
        for k in range(HC):
            t = const.tile([128, H], BF, tag=f"wh{k}", name=f"wh_sb{k}")
            nc.sync.dma_start(out=t, in_=wh[k * 128:(k + 1) * 128, :])
            wh_sb.append(t)
        at_sb = []
        for m in range(HC):
            t = const.tile([128, bpc], FP, tag=f"at{m}", name=f"at{m}")
            nc.sync.dma_start(out=t, in_=at[m * 128:(m + 1) * 128, :])
            at_sb.append(t)
        vwt_sb = const.tile([128, HC], BF, tag="vwt")
        nc.sync.dma_start(out=vwt_sb, in_=vwt[:, :])

        e_sb = const.tile([bpc, s], FP, tag="e_sb")

        # deferred e-dot matmuls: flushed one group late so their f
        # inputs are ready when PE reaches them (no head-of-line stall)
        pend = []

        def flush(item):
            fs, bb, gg = item
            ps_e = pse_pool.tile([1, SLAB], FP, tag="ps_e")
            for m in range(HC):
                nc.tensor.matmul(
                    ps_e,
                    vwt_sb[:, m:m + 1],
                    fs[m][:, :],
                    start=(m == 0),
                    stop=(m == HC - 1),
                )
            e_g = egpool.tile([1, SLAB], FP, tag="e_g")
            nc.vector.tensor_copy(e_g, ps_e)
            nc.sync.dma_start(
                out=e_sb[bb:bb + 1, gg * SLAB:(gg + 1) * SLAB], in_=e_g)

        # ---------------- main loop ----------------
        for b in range(bpc):
            for g in range(NG):
                src = xt[b * H:(b + 1) * H, g * SLAB:(g + 1) * SLAB]
                src = src.rearrange("(k p) s -> p k s", p=128)
                xs = xpool.tile([128, HC, SLAB], BF, tag="xs")
                nc.sync.dma_start(out=xs, in_=src)

                fs = []
                for m in range(HC):
                    ms = slice(m * 128, (m + 1) * 128)
                    ps = psf_pool.tile([128, SLAB], FP, tag="ps_f")
                    for k in range(HC):
                        nc.tensor.matmul(
                            ps,
                            wh_sb[k][:, ms],
                            xs[:, k, :],
                            start=(k == 0),
                            stop=(k == HC - 1),
                        )
                    f_m = fpool.tile([128, SLAB], BF, tag="f_m")
                    nc.scalar.activation(
                        out=f_m, in_=ps,
                        func=mybir.ActivationFunctionType.Tanh,
                        bias=at_sb[m][:, b:b + 1],
                    )
                    fs.append(f_m)
                pend.append((fs, b, g))
                if len(pend) > 1:
                    flush(pend.pop(0))
        while pend:
            flush(pend.pop(0))

        # ---------------- softmax + output (v1 pattern) ----------------
        smx = const.tile([bpc, 1], FP, tag="smx")
        nc.vector.tensor_reduce(
            out=smx, in_=e_sb, axis=mybir.AxisListType.X,
            op=mybir.AluOpType.max, negate=True,
        )
        p_sb = const.tile([bpc, s], FP, tag="p_sb")
        esum = const.tile([bpc, 1], FP, tag="esum")
        nc.scalar.activation(
            out=p_sb, in_=e_sb, func=mybir.ActivationFunctionType.Exp,
            bias=smx, accum_out=esum,
        )
        rsum = const.tile([bpc, 1], FP, tag="rsum")
        nc.vector.reciprocal(rsum, esum)
        a_out = const.tile([bpc, s], FP, tag="a_out")
        nc.vector.tensor_scalar_mul(a_out, p_sb, rsum)
        nc.sync.dma_start(out=out_a[:, :], in_=a_out)

    return nc


_PROG_CACHE = {}


def _get_program(key=(BPC, S)):
    if key not in _PROG_CACHE:
        nc = build_program(*key)
        nc.finalize()
        _PROG_CACHE[key] = nc
    return _PROG_CACHE[key]


def _to_bf16_u16(a):
    """Round-to-nearest-even fp32 -> bf16 bit pattern (uint16)."""
    u = np.ascontiguousarray(a, dtype=np.float32).view(np.uint32)
    return ((u + 0x7FFF + ((u >> 16) & 1)) >> 16).astype(np.uint16)


def make_in_maps(encoder_output, decoder_hidden, coverage, Wh, bh, Ws, bs, Wc, bc,
                 v_w, v_b=None):
    f32 = np.float32
    enc = np.asarray(encoder_output, dtype=f32)
    cov = np.asarray(coverage, dtype=f32)
    Wh64 = np.asarray(Wh, dtype=np.float64)
    # u @ Wh == Wc[0] exactly (f64 solve) -> coverage folds into x
    u = np.linalg.solve(Wh64.T, np.asarray(Wc, dtype=np.float64)[0])
    A = (np.asarray(decoder_hidden, dtype=np.float64)
         @ np.asarray(Ws, dtype=np.float64)
         + np.asarray(bh, dtype=np.float64)
         + np.asarray(bs, dtype=np.float64)
         + np.asarray(bc, dtype=np.float64)).astype(f32)  # [B, H]

    vw = np.asarray(v_w, dtype=f32).reshape(HC, 128)      # chunk m -> col m
    shared = {
        "wh": _to_bf16_u16(np.asarray(Wh, dtype=f32)),
        "vwt": _to_bf16_u16(np.ascontiguousarray(vw.T)),  # [128, HC]
    }
    uf = u.astype(f32)
    in_maps = []
    for c in range(N_CORES):
        lo, hi = c * BPC, (c + 1) * BPC
        xf = enc[lo:hi] + cov[lo:hi][:, :, None] * uf
        xtc = np.ascontiguousarray(xf.transpose(0, 2, 1)).reshape(BPC * H, S)
        m = dict(shared)
        m["xt"] = _to_bf16_u16(xtc)
        m["at"] = np.ascontiguousarray(A[lo:hi].T)        # [H, bpc]
        in_maps.append(m)
    return in_maps


def run_spmd(in_maps, trace=False, **kw):
    from concourse.bass_utils import run_bass_kernel_spmd
    nc = _get_program()
    return run_bass_kernel_spmd(nc, in_maps, core_ids=list(range(N_CORES)),
                                trace=trace, **kw)


def kernel(**inputs) -> tuple[np.ndarray, np.ndarray]:
    in_maps = make_in_maps(**inputs)
    res = run_spmd(in_maps)
    a_t = np.concatenate([r["out_a"] for r in res.results], axis=0)
    a_t = a_t.astype(np.float32)
    cov = np.asarray(inputs["coverage"], dtype=np.float32)
    return a_t, cov + a_t
